# revision 1
# baseline (speedup 1.0000x reference)
"""NTXent contrastive loss on 8 Trainium2 NeuronCores (Bass/Tile).

Math: with z = rows of x normalized (zh), all four cosine-sim matrices are
blocks of the single gram G = zh @ zh.T over the 8192 rows.  The reference's
upper-triangle masked sum collapses algebraically to full-matrix sums:

    sim_all = 0.5 * S_total + n*e^0.5 + sim_s
    S_total = sum_{ij} exp(G_ij / 2)            (8192 x 8192)
    sim_s   = sum_i exp(cos(p_i, q_i) / 2)      (i = 0..n-1, q_i = row i+n)
    loss    = -log(sim_s / sim_all)

Sharding: the 16x16 grid of 512x512 G-blocks (upper block triangle incl.
diagonal = 136 blocks) is covered exactly once by giving core c the blocks
{(i, i+d mod 16): i in {c, c+8}, d=0..7} u {(c, c+8)}.  After cyclically
rolling the input rows by -512*c for core c, every core runs the IDENTICAL
program computing canonical blocks {(0,0..8), (8,8..15)} — a uniform SPMD
kernel with no collectives.  S_total = 2*U - Dblk where U is the sum over
computed blocks and Dblk the sum over the two diagonal blocks per core.

Per-core pipeline: DMA x (8MB) -> sumsq (DVE) -> 1/norm via exp(-0.5*ln) (ACT)
-> normalize+cast bf16 (DVE) -> PE transpose -> PSUM->SBUF copy (ACT/DVE) ->
bf16 gram matmuls (PE, fp32 PSUM) -> fused exp+row-sum (ACT accum_out).
Device outputs are 18 partial-sum columns [128,18]; host reduces in f64.
"""

import sys

for _p in ("/opt/trn_rl_repo", "/root/.axon_site"):
    if _p not in sys.path:
        sys.path.insert(0, _p)

import numpy as np

P = 128          # partitions
D = 256          # feature dim
N = 8192         # total rows
BAND = 512       # gram block edge
NCORES = 8
NCHUNK = 8       # x is loaded in 8 chunks of 1024 rows
TPC = 8          # 128-row tiles per chunk
NRT = 64         # 128-row tiles total
# canonical gram blocks (band-pairs) per core, in emission order
BLOCKS = ([(0, j) for j in range(4)] + [(0, j) for j in range(4, 8)]
          + [(8, 8), (0, 8), (8, 9), (8, 10), (8, 11)]
          + [(8, j) for j in range(12, 16)])
DIAG_IDX = (0, 8)   # indices of (0,0) and (8,8) in BLOCKS
NBLK = len(BLOCKS)  # 17
SIMS_COL = NBLK     # acc column holding the sim_s partial
ACC_COLS = NBLK + 1
ACT_SUMSQ = 3       # of each 8-tile chunk, how many sumsq tiles go to ACT

_PROG = None  # cached (nc, input name, output name)


def _build_program():
    import concourse.bacc as bacc
    import concourse.mybir as mybir
    from concourse import tile
    from concourse.masks import make_identity

    f32 = mybir.dt.float32
    bf16 = mybir.dt.bfloat16
    AF = mybir.ActivationFunctionType
    ALU = mybir.AluOpType

    nc = bacc.Bacc("TRN2", target_bir_lowering=False, debug=False,
                   num_devices=NCORES)
    x_d = nc.dram_tensor("x", [N, D], f32, kind="ExternalInput")
    acc_d = nc.dram_tensor("acc", [P, ACC_COLS], f32, kind="ExternalOutput")

    with tile.TileContext(nc) as tc:
        with (
            tc.tile_pool(name="consts", bufs=1) as consts,
            tc.tile_pool(name="xch", bufs=3) as xch,
            tc.tile_pool(name="zh", bufs=6) as zhp,
            tc.tile_pool(name="zhT", bufs=1) as zhtp,
            tc.tile_pool(name="stats", bufs=1) as stats,
            tc.tile_pool(name="scr", bufs=2) as scr,
            tc.tile_pool(name="escr", bufs=2) as escrp,
            tc.tile_pool(name="psum", bufs=2, space="PSUM") as psum,
        ):
            ident = consts.tile([P, P], bf16, tag="ident")
            make_identity(nc, ident[:])

            sumsq = stats.tile([P, NRT], f32, tag="sumsq")
            lntile = stats.tile([P, NRT], f32, tag="ln")
            rn = stats.tile([P, NRT], f32, tag="rn")
            acc = stats.tile([P, ACC_COLS], f32, tag="acc")
            dots = stats.tile([P, 4], f32, tag="dots")
            rnp = stats.tile([P, 4], f32, tag="rnp")
            dots2 = stats.tile([P, 4], f32, tag="dots2")
            scr4 = stats.tile([P, 4], f32, tag="scr4")
            xp_keep = stats.tile([P, 4, D], f32, tag="xpk")

            # zhT[k][g]: [128, 2048] bf16, k = feature half, g = 4-band group
            zht = [[zhtp.tile([P, 4 * BAND], bf16, tag=f"zhT{k}{g}",
                              name=f"zhT{k}{g}")
                    for g in range(4)] for k in range(2)]

            # transposed-chunk PSUM staging tiles in flight, keyed (k, g)
            tpsum = {}
            xt4 = None  # chunk-4 SBUF tile, reused for sim_s q rows

            def emit_block(bidx):
                bi, bj = BLOCKS[bidx]
                pt = psum.tile([P, 4 * BAND], f32, tag="ps")
                for m in range(4):
                    for k in range(2):
                        nc.tensor.matmul(
                            pt[:, m * BAND:(m + 1) * BAND],
                            zht[k][bi // 4][:, (bi % 4) * BAND + m * P:
                                            (bi % 4) * BAND + (m + 1) * P],
                            zht[k][bj // 4][:, (bj % 4) * BAND:
                                            (bj % 4 + 1) * BAND],
                            start=(k == 0), stop=(k == 1),
                        )
                et = escrp.tile([P, 4 * BAND], bf16, tag="escr")
                nc.scalar.activation(et[:], pt[:], AF.Exp, scale=0.5,
                                     accum_out=acc[:, bidx:bidx + 1])

            # blocks emitted once their zhT groups are complete
            ready = {1: [0, 1, 2, 3], 3: [4, 5, 6, 7],
                     5: [8, 9, 10, 11, 12], 7: [13, 14, 15, 16]}

            for j in range(NCHUNK):
                xt = xch.tile([P, TPC, D], f32, tag="xc")
                nc.sync.dma_start(
                    xt[:],
                    x_d[1024 * j:1024 * (j + 1), :]
                    .rearrange("(t p) d -> p t d", p=P),
                )
                if j == 4:
                    xt4 = xt
                g = j // 2
                if j % 2 == 0:
                    for k in range(2):
                        tpsum[(k, g)] = psum.tile([P, 4 * BAND], bf16,
                                                  tag="ps", name=f"tp{k}{g}")
                # squares on the otherwise-idle GPSIMD, one 3D op per chunk;
                # one batched DVE reduce -> sumsq[:, 8j:8j+8]
                sq = scr.tile([P, TPC, D], f32, tag="sq")
                nc.gpsimd.tensor_tensor(out=sq[:], in0=xt[:], in1=xt[:],
                                        op=ALU.mult)
                nc.vector.tensor_reduce(
                    out=sumsq[:, TPC * j:TPC * (j + 1)], in_=sq[:],
                    axis=mybir.AxisListType.X, op=ALU.add)
                # 1/norm = exp(-0.5 * ln(sumsq)); keeps ACT in one table set
                sl = slice(TPC * j, TPC * (j + 1))
                nc.scalar.activation(lntile[:, sl], sumsq[:, sl], AF.Ln)
                nc.scalar.activation(rn[:, sl], lntile[:, sl], AF.Exp,
                                     scale=-0.5)
                for t in range(TPC):
                    r = TPC * j + t
                    zt = zhp.tile([P, D], bf16, tag="zh")
                    nc.vector.tensor_scalar_mul(zt[:], xt[:, t, :],
                                                rn[:, r:r + 1])
                    for k in range(2):
                        nc.tensor.transpose(
                            tpsum[(k, g)][:, (r % 16) * P:(r % 16 + 1) * P],
                            zt[:, k * P:(k + 1) * P],
                            ident[:],
                        )
                if j % 2 == 1:
                    # bf16 psum->sbuf: DVE gets the 2-byte 2x copy mode
                    for k in range(2):
                        nc.vector.tensor_copy(zht[k][g][:], tpsum[(k, g)][:])
                        del tpsum[(k, g)]

                if j == 4:
                    # sim_s: permuted rows 0..511 vs 4096..4607 (= chunk 4)
                    nc.sync.dma_start(
                        xp_keep[:],
                        x_d[0:1024, :].rearrange("(t p) d -> p t d", p=P)
                        [:, 0:4, :],
                    )
                    st = scr.tile([P, 4, D], f32, tag="sq")
                    nc.gpsimd.tensor_tensor(out=st[:], in0=xp_keep[:],
                                            in1=xt4[:, 0:4, :], op=ALU.mult)
                    nc.vector.tensor_reduce(
                        out=dots[:], in_=st[:],
                        axis=mybir.AxisListType.X, op=ALU.add)
                    nc.vector.tensor_mul(rnp[:], rn[:, 0:4], rn[:, 32:36])
                    nc.vector.tensor_mul(dots2[:], dots[:], rnp[:])
                    nc.scalar.activation(scr4[:], dots2[:], AF.Exp, scale=0.5,
                                         accum_out=acc[:, SIMS_COL:
                                                       SIMS_COL + 1])

                for bidx in ready.get(j, []):
                    emit_block(bidx)

            nc.sync.dma_start(acc_d[:], acc[:])

    nc.compile()
    return nc


def _get_prog():
    global _PROG
    if _PROG is None:
        _PROG = _build_program()
    return _PROG


def run_device(x, trace=False, tmpdir=None):
    """Run the SPMD program; returns (per-core acc arrays, BassKernelResults)."""
    from concourse.bass_utils import run_bass_kernel_spmd

    if trace:
        _install_ntff_hook()
    nc = _get_prog()
    in_maps = [{"x": np.ascontiguousarray(np.roll(x, -BAND * c, axis=0))}
               for c in range(NCORES)]
    res = run_bass_kernel_spmd(nc, in_maps, list(range(NCORES)),
                               trace=trace, tmpdir=tmpdir)
    accs = [res.results[c]["acc"] for c in range(NCORES)]
    return accs, res


def _install_ntff_hook():
    """The agent image lacks antenv.axon_hooks; inject the ctypes-based
    NTFF profiling hook so run_bass_kernel_spmd(trace=True) works."""
    import types

    if "antenv.axon_hooks" in sys.modules:
        return
    try:
        from trn_agent_boot.trn_boot import _ntff_profile_via_ctypes
        hook = _ntff_profile_via_ctypes("/opt/axon/libaxon_pjrt.so")
    except Exception:
        hook = None
    mod = types.ModuleType("antenv.axon_hooks")
    mod.get_axon_ntff_profile_hook = lambda: hook
    mod.set_axon_ntff_profile_hook = lambda h: None
    sys.modules["antenv.axon_hooks"] = mod


def combine(accs):
    """Host-side unshard: fold per-core partial sums into the scalar loss."""
    U = 0.0
    Dblk = 0.0
    sims = 0.0
    for a in accs:
        a = a.astype(np.float64)
        U += a[:, :NBLK].sum()
        Dblk += a[:, DIAG_IDX[0]].sum() + a[:, DIAG_IDX[1]].sum()
        sims += a[:, SIMS_COL].sum()
    S_total = 2.0 * U - Dblk
    sim_all = 0.5 * S_total + (N // 2) * np.exp(0.5) + sims
    return np.array(-np.log(sims / sim_all), dtype=np.float32)


def kernel(x, unused=None, **_ignored):
    x = np.asarray(x, dtype=np.float32)
    accs, _ = run_device(x, trace=False)
    return combine(accs)


if __name__ == "__main__":
    rng = np.random.default_rng(0)
    x = rng.standard_normal((N, D)).astype(np.float32)
    print(kernel(x))

